# revision 17
# baseline (speedup 1.0000x reference)
"""Causal depthwise temporal conv (K=4) on 8 TRN2 NeuronCores.

Reference semantics (for x: [B, T, D], w: [K, D], b: [D]):
    out[bt, t, d] = sum_{j=0}^{K-1} x_pad[bt, t + j, d] * w[j, d] + b[d]
where x_pad is x left-padded with K-1 zeros along time.

Strategy:
  - Tensor-parallel over the channel axis: core m owns channels
    [m*512, (m+1)*512) -- the conv is depthwise so channels are fully
    independent (no collectives).
  - Host pre-transposes each core's shard to channel-major [D_sh, B, T+K-1]
    (left zero-padded). On device, channels sit on SBUF partitions so the
    per-channel weight becomes a per-partition scalar, and the temporal
    shifts become free-dimension slices.
  - Per (channel-block, batch): one ACT op computes w0*x + b, then three
    DVE scalar_tensor_tensor ops each fuse (x_shift * w_j) + acc.
    All DMAs are large contiguous HWDGE transfers.
"""

import numpy as np

import concourse.bacc as bacc
import concourse.mybir as mybir
from concourse.tile import TileContext
from concourse import bass_utils

B = 4            # batch
T = 4096         # sequence length
D = 4096         # channels (width)
K = 4            # temporal taps
N_CORES = 8
D_SH = D // N_CORES          # 512 channels per core
P = 128                      # SBUF partitions
N_BLK = D_SH // P            # 4 channel blocks per core
TP = T + K - 1               # padded time length


def _build(b=B, t=T, n_blk=N_BLK, batch_pair=2):
    nc = bacc.Bacc("TRN2")
    tp = t + K - 1
    f32 = mybir.dt.float32
    x = nc.dram_tensor("x", [n_blk, P, b, tp], f32, kind="ExternalInput")
    wb = nc.dram_tensor("wb", [n_blk, P, K + 1], f32, kind="ExternalInput")
    out = nc.dram_tensor("out", [n_blk, P, b, t], f32, kind="ExternalOutput")
    mult, add = mybir.AluOpType.mult, mybir.AluOpType.add
    ident_fn = mybir.ActivationFunctionType.Identity

    with TileContext(nc) as tc:
        with tc.tile_pool(name="pool", bufs=2) as pool:
            for blk in range(n_blk):
                wt = pool.tile([P, K + 1], f32, tag="wb")
                nc.sync.dma_start(wt[:], wb[blk])
                for b0 in range(0, b, batch_pair):
                    nb = min(batch_pair, b - b0)
                    X = pool.tile([P, nb, tp], f32, tag="x")
                    nc.sync.dma_start(X[:], x[blk, :, b0:b0 + nb, :])
                    for bi in range(nb):
                        # Per-batch chain, ping-pong accumulators:
                        # ACT does w0*x0+b, DVE does 3 fused FMAs.
                        a0 = pool.tile([P, t], f32, tag="accA")
                        nc.scalar.activation(a0[:], X[:, bi, 0:t], ident_fn,
                                             bias=wt[:, K:K + 1],
                                             scale=wt[:, 0:1])
                        a1 = pool.tile([P, t], f32, tag="accB")
                        nc.vector.scalar_tensor_tensor(
                            a1[:], X[:, bi, 1:1 + t], wt[:, 1:2], a0[:],
                            mult, add)
                        a2 = pool.tile([P, t], f32, tag="accA")
                        nc.vector.scalar_tensor_tensor(
                            a2[:], X[:, bi, 2:2 + t], wt[:, 2:3], a1[:],
                            mult, add)
                        a3 = pool.tile([P, t], f32, tag="accB")
                        nc.vector.scalar_tensor_tensor(
                            a3[:], X[:, bi, 3:3 + t], wt[:, 3:4], a2[:],
                            mult, add)
                        nc.sync.dma_start(out[blk, :, b0 + bi, :], a3[:])
    nc.compile()
    return nc


def _prepare(x, w, b):
    x = np.asarray(x, dtype=np.float32)
    w = np.asarray(w, dtype=np.float32)
    b = np.asarray(b, dtype=np.float32)
    # channel-major, left zero-padded time: [D, B, TP]
    xp = np.zeros((D, B, TP), dtype=np.float32)
    xp[:, :, K - 1:] = x.transpose(2, 0, 1)
    wbt = np.concatenate([w.T, b[:, None]], axis=1).astype(np.float32)  # [D, K+1]
    in_maps = []
    for m in range(N_CORES):
        sl = slice(m * D_SH, (m + 1) * D_SH)
        in_maps.append({
            "x": np.ascontiguousarray(xp[sl]).reshape(N_BLK, P, B, TP),
            "wb": np.ascontiguousarray(wbt[sl]).reshape(N_BLK, P, K + 1),
        })
    return in_maps


def _collect(results):
    out = np.empty((B, T, D), dtype=np.float32)
    for m in range(N_CORES):
        o = np.asarray(results[m]["out"]).reshape(D_SH, B, T)
        out[:, :, m * D_SH:(m + 1) * D_SH] = o.transpose(1, 2, 0)
    return out


def _run(in_maps, trace=False, **kwargs):
    nc = _build()
    return bass_utils.run_bass_kernel_spmd(
        nc, in_maps, core_ids=list(range(N_CORES)), trace=trace, **kwargs)


def kernel(x, w, b):
    in_maps = _prepare(x, w, b)
    try:
        res = _run(in_maps)
    except Exception:
        # Transient NRT device errors have been observed on a cold first
        # execute; one retry (fresh compile dir) clears them.
        res = _run(in_maps)
    return _collect(res.results)


# revision 18
# speedup vs baseline: 1.0213x; 1.0213x over previous
"""Causal depthwise temporal conv (K=4) on 8 TRN2 NeuronCores.

Reference semantics (for x: [B, T, D], w: [K, D], b: [D]):
    out[bt, t, d] = sum_{j=0}^{K-1} x_pad[bt, t + j, d] * w[j, d] + b[d]
where x_pad is x left-padded with K-1 zeros along time.

Strategy:
  - Tensor-parallel over the channel axis: core m owns channels
    [m*512, (m+1)*512) -- the conv is depthwise so channels are fully
    independent (no collectives).
  - Host pre-transposes each core's shard to channel-major [D_sh, B, T+K-1]
    (left zero-padded). On device, channels sit on SBUF partitions so the
    per-channel weight becomes a per-partition scalar, and the temporal
    shifts become free-dimension slices.
  - Per (channel-block, batch): one ACT op computes w0*x + b, then three
    DVE scalar_tensor_tensor ops each fuse (x_shift * w_j) + acc.
    All DMAs are large contiguous HWDGE transfers.
"""

import numpy as np

import concourse.bacc as bacc
import concourse.mybir as mybir
from concourse.tile import TileContext
from concourse import bass_utils

B = 4            # batch
T = 4096         # sequence length
D = 4096         # channels (width)
K = 4            # temporal taps
N_CORES = 8
D_SH = D // N_CORES          # 512 channels per core
P = 128                      # SBUF partitions
N_BLK = D_SH // P            # 4 channel blocks per core
TP = T + K - 1               # padded time length


def _build(b=B, t=T, n_blk=N_BLK, batch_pair=2):
    nc = bacc.Bacc("TRN2")
    tp = t + K - 1
    f32 = mybir.dt.float32
    x = nc.dram_tensor("x", [n_blk, P, b, tp], f32, kind="ExternalInput")
    wb = nc.dram_tensor("wb", [n_blk, P, K + 1], f32, kind="ExternalInput")
    out = nc.dram_tensor("out", [n_blk, P, b, t], f32, kind="ExternalOutput")
    mult, add = mybir.AluOpType.mult, mybir.AluOpType.add
    ident_fn = mybir.ActivationFunctionType.Identity

    with TileContext(nc) as tc:
        with tc.tile_pool(name="pool", bufs=4) as pool, \
             tc.tile_pool(name="poola", bufs=3) as poola:
            for blk in range(n_blk):
                wt = pool.tile([P, K + 1], f32, tag="wb")
                nc.sync.dma_start(wt[:], wb[blk])
                for bb in range(b):
                    # Per-batch loads (2.1MB) shorten the pipeline ramp;
                    # bufs=4 keeps several loads in flight.
                    X = pool.tile([P, tp], f32, tag="x")
                    nc.sync.dma_start(X[:], x[blk, :, bb, :])
                    # Per-batch chain, ping-pong accumulators:
                    # ACT does w0*x0+b, DVE does 3 fused FMAs.
                    a0 = poola.tile([P, t], f32, tag="accA")
                    nc.scalar.activation(a0[:], X[:, 0:t], ident_fn,
                                         bias=wt[:, K:K + 1],
                                         scale=wt[:, 0:1])
                    a1 = poola.tile([P, t], f32, tag="accB")
                    nc.vector.scalar_tensor_tensor(
                        a1[:], X[:, 1:1 + t], wt[:, 1:2], a0[:],
                        mult, add)
                    a2 = poola.tile([P, t], f32, tag="accA")
                    nc.vector.scalar_tensor_tensor(
                        a2[:], X[:, 2:2 + t], wt[:, 2:3], a1[:],
                        mult, add)
                    a3 = poola.tile([P, t], f32, tag="accB")
                    nc.vector.scalar_tensor_tensor(
                        a3[:], X[:, 3:3 + t], wt[:, 3:4], a2[:],
                        mult, add)
                    nc.sync.dma_start(out[blk, :, bb, :], a3[:])
    nc.compile()
    return nc


def _prepare(x, w, b):
    x = np.asarray(x, dtype=np.float32)
    w = np.asarray(w, dtype=np.float32)
    b = np.asarray(b, dtype=np.float32)
    # channel-major, left zero-padded time: [D, B, TP]
    xp = np.zeros((D, B, TP), dtype=np.float32)
    xp[:, :, K - 1:] = x.transpose(2, 0, 1)
    wbt = np.concatenate([w.T, b[:, None]], axis=1).astype(np.float32)  # [D, K+1]
    in_maps = []
    for m in range(N_CORES):
        sl = slice(m * D_SH, (m + 1) * D_SH)
        in_maps.append({
            "x": np.ascontiguousarray(xp[sl]).reshape(N_BLK, P, B, TP),
            "wb": np.ascontiguousarray(wbt[sl]).reshape(N_BLK, P, K + 1),
        })
    return in_maps


def _collect(results):
    out = np.empty((B, T, D), dtype=np.float32)
    for m in range(N_CORES):
        o = np.asarray(results[m]["out"]).reshape(D_SH, B, T)
        out[:, :, m * D_SH:(m + 1) * D_SH] = o.transpose(1, 2, 0)
    return out


def _run(in_maps, trace=False, **kwargs):
    nc = _build()
    return bass_utils.run_bass_kernel_spmd(
        nc, in_maps, core_ids=list(range(N_CORES)), trace=trace, **kwargs)


def kernel(x, w, b):
    in_maps = _prepare(x, w, b)
    try:
        res = _run(in_maps)
    except Exception:
        # Transient NRT device errors have been observed on a cold first
        # execute; one retry (fresh compile dir) clears them.
        res = _run(in_maps)
    return _collect(res.results)
